# revision 1
# baseline (speedup 1.0000x reference)
"""Trainium2 Bass kernel for nn_CameraOptimizer.

Math: per camera n, the reference computes
    R_corr = rodrigues(rotation_deltas[n]) @ noisy_R[n]
    t_corr = noisy_t[n] + translation_deltas[n]
    fx,fy,cx,cy = noisy_K[n] + intrinsic_deltas[n]
and per point b (cam = camera_indices[b], X = X_world[b]):
    Xc = R_corr @ X + t_corr;  u = fx*Xc0/z + cx;  v = fy*Xc1/z + cy;  z = Xc2
which folds into a projective form with 12 per-camera coefficients:
    u = (B0.X + b0) / (R2.X + t2),  B0 = fx*R0 + cx*R2, b0 = fx*t0 + cx*t2
    v = (B1.X + b1) / (R2.X + t2),  B1 = fy*R1 + cy*R2, b1 = fy*t1 + cy*t2

Sharding strategy: data-parallel over points across the 8 cores. Within each
core's shard, points are grouped by camera (host-side layout prep, part of
the sharding step) so that on device every SBUF partition processes exactly
one camera per round — the 12 camera coefficients become per-partition
scalars read by tensor_scalar / scalar_tensor_tensor / activation ops, and
the kernel is a pure streaming elementwise pipeline at the memory roofline.
No per-point gather is needed on device (per-point indexed gathers are far
off the roofline on TRN2: Q7 ap_gather is command-rate-bound and dma_gather
needs 256B records).

Layout per core: 8 rounds x (128,...,128,104) partitions = 1000 camera
slots, cameras sorted by per-core point count so the per-round column width
S_r is tight (few % padding). Inputs are packed planar:
xp[round][plane c][partition][S_r]; outputs come back as
op[round][u|v plane][partition][S_r] and are scattered back to the original
point order on the host.

Engine split per 3-term dot product chain: head mul+add on ACT
(activation Identity with per-partition scale/bias), the two fused
mul+adds on DVE (scalar_tensor_tensor); 1/z = exp(-ln z) on ACT (one
table set, ~2 ULP); the final two tensor_tensor multiplies on GPSIMD
(plain TensorTensor — the only per-partition-data op walrus allows on
Pool; TensorScalarPtr is rejected there).
"""

import numpy as np

import concourse.bass as bass
import concourse.mybir as mybir
import concourse.tile as tile
from concourse.bass_utils import run_bass_kernel_spmd

NUM_CAMERAS = 1000
NCORES = 8
ROUNDS = 8
P = 128
# cameras per round: 7*128 + 104 = 1000 (no dummy slots)
P_LIST = [P] * (ROUNDS - 1) + [NUM_CAMERAS - (ROUNDS - 1) * P]

F32 = mybir.dt.float32

# Results of the last device run (exec_time_ns etc.) for test harnesses.
LAST_RESULTS = None
LAST_S_LIST = None

# Engine/pipeline configuration chosen via cost-model timeline search (see
# _build_program docstring): heads on ACT, fused mul-adds on DVE, final
# multiplies on GPSIMD, 1/z = exp(-ln z) on ACT, outputs on the SP ring
# behind the hoisted input stream.
BEST_CONFIG = dict(mid_engine="dve", recip_mode="expln", z_mid_engine="dve",
                   out_engine="sync", coef_engine="scalar", bufs=(10, 6),
                   u_mul_engine="pool", v_mul_engine="pool",
                   head_mode="half", tail_mode="half", last_mul_dve=True)


def _legalize_single_wait(nc):
    """Hoist extra sync waits onto same-engine NOPs.

    The walrus build staged here rejects any instruction carrying more than
    one sync wait ("Too many sync wait commands"). Engines dispatch in order,
    so a chain of single-wait NOPs in front of the instruction is equivalent.
    """
    f = nc.m.functions[0]
    for bb in f.blocks:
        out = []
        for inst in list(bb.instructions):
            si = inst.sync_info
            waits = list(si.on_wait) if (si is not None and si.on_wait) else []
            if len(waits) > 1:
                builder = nc.engines[inst.engine]
                for w in waits[:-1]:
                    nop_inst = builder.nop(nofuse=True).ins
                    for bb2 in f.blocks:
                        if bb2.instructions and bb2.instructions[-1] is nop_inst:
                            bb2.instructions.pop()
                            break
                    nop_inst.sync_info = mybir.SyncInfo(on_wait=[w], on_update=[])
                    out.append(nop_inst)
                si.on_wait = [waits[-1]]
            out.append(inst)
        bb.instructions[:] = out


def _camera_coeffs(noisy_K, noisy_R, noisy_t, intrinsic_deltas, rotation_deltas,
                   translation_deltas):
    """[N,12] float32 projective coefficients per camera (computed in f64)."""
    w = rotation_deltas.astype(np.float64)
    theta = np.linalg.norm(w, axis=-1, keepdims=True)
    k = w / np.maximum(theta, 1e-8)
    kx, ky, kz = k[:, 0], k[:, 1], k[:, 2]
    zero = np.zeros_like(kx)
    K = np.stack(
        [zero, -kz, ky, kz, zero, -kx, -ky, kx, zero], axis=-1
    ).reshape(-1, 3, 3)
    I = np.eye(3, dtype=np.float64)
    s = np.sin(theta)[..., None]
    c = np.cos(theta)[..., None]
    R_delta = I + s * K + (1.0 - c) * (K @ K)
    R = R_delta @ noisy_R.astype(np.float64)
    t = (noisy_t + translation_deltas).astype(np.float64)
    Kc = (noisy_K + intrinsic_deltas).astype(np.float64)
    fx, fy, cx, cy = Kc[:, 0], Kc[:, 1], Kc[:, 2], Kc[:, 3]

    n = R.shape[0]
    C = np.empty((n, 12), np.float64)
    C[:, 0:3] = fx[:, None] * R[:, 0, :] + cx[:, None] * R[:, 2, :]
    C[:, 3] = fx * t[:, 0] + cx * t[:, 2]
    C[:, 4:7] = fy[:, None] * R[:, 1, :] + cy[:, None] * R[:, 2, :]
    C[:, 7] = fy * t[:, 1] + cy * t[:, 2]
    C[:, 8:11] = R[:, 2, :]
    C[:, 11] = t[:, 2]
    return C.astype(np.float32)


def _build_program(S_list, mid_engine="pool", recip_mode="expln",
                   z_mid_engine="dve", bufs=4, out_engine="vector",
                   coef_engine="scalar", u_tail_engine="dve",
                   v_tail_engine="dve", z_tail_engine="dve",
                   u_mul_engine="dve", v_mul_engine="dve",
                   head_mode="half", tail_mode="half", last_mul_dve=False,
                   first_chunk_split_rings=False, compute_mode="dve",
                   first_chunk_swdge=False):
    """One-core Bass/Tile program, SPMD across 8 cores with per-core data.

    mid_engine: engine for the middle mul-add of the u/v dot products
        ("pool" offloads it to GPSIMD, "dve" keeps it on VectorE).
    recip_mode: "expln" computes 1/z as exp(-ln z) on the ACT engine;
        "dve" uses the stock DVE iterative-divide reciprocal.
    z_mid_engine: engine for the middle mul-add of the z chain.
    """
    totx = sum(3 * pr * S for pr, S in zip(P_LIST, S_list))
    toto = sum(2 * pr * S for pr, S in zip(P_LIST, S_list))

    nc = bass.Bass()
    xp = nc.dram_tensor("xp", [totx], F32, kind="ExternalInput")
    coef = nc.dram_tensor("coef", [ROUNDS * P, 16], F32, kind="ExternalInput")
    op = nc.dram_tensor("op", [toto], F32, kind="ExternalOutput")

    mult = mybir.AluOpType.mult
    add = mybir.AluOpType.add
    ident = mybir.ActivationFunctionType.Identity

    if isinstance(bufs, int):
        xbufs = obufs = bufs
    else:
        xbufs, obufs = bufs
    with tile.TileContext(nc) as tc:
        with (
            tc.tile_pool(name="cpool", bufs=1) as cpool,
            tc.tile_pool(name="xpool", bufs=xbufs) as xpool,
            tc.tile_pool(name="pool", bufs=obufs) as pool,
        ):
            # Warm up the ACT table set (natural_log_exp_and_others, which
            # also contains identity) before any data arrives, so the ~2.7us
            # ACT_TABLE_LOAD overlaps the first input DMAs instead of sitting
            # on the critical path.
            warm = cpool.tile([P, 8], F32)
            nc.gpsimd.memset(warm[:], 1.0)
            nc.scalar.activation(out=warm[:], in_=warm[:],
                                 func=mybir.ActivationFunctionType.Ln)

            mid = nc.gpsimd if mid_engine == "pool" else nc.vector
            zmid = nc.gpsimd if z_mid_engine == "pool" else nc.vector

            # Tiny coefficient DMA first — it gates every compute op. Issued
            # on the ACT HWDGE ring so it runs in parallel with the round-0
            # x-plane DMAs on the SP ring.
            ctile = cpool.tile([P, ROUNDS * 16], F32)
            getattr(nc, coef_engine).dma_start(
                out=ctile[:].rearrange("p (r c) -> p r c", r=ROUNDS),
                in_=coef.ap().rearrange("(r p) c -> p r c", p=P),
            )

            # Column chunks: first and last rounds split in half (shorter
            # pipeline fill/drain); every chunk capped at 768 columns so a
            # pathological camera-count skew cannot blow out SBUF tiles.
            CAP = 512 if compute_mode == "pe" else 768
            xbases = []
            obases = []
            xoff = 0
            ooff = 0
            for r in range(ROUNDS):
                xbases.append(xoff)
                obases.append(ooff)
                xoff += 3 * P_LIST[r] * S_list[r]
                ooff += 2 * P_LIST[r] * S_list[r]
            chunks = []
            for r in range(ROUNDS):
                S = S_list[r]
                if compute_mode == "pe":
                    n = -(-S // CAP)
                    w = ((-(-S // n)) + 3) // 4 * 4
                    chunks.extend(
                        (r, i * w, min((i + 1) * w, S)) for i in range(n)
                    )
                    continue
                if r == 0 and S <= CAP:
                    if head_mode == "tri":
                        h0 = min(160, S // 2)
                        h1 = h0 + (((S - h0) // 2 + 3) & ~3)
                        parts = [(0, h0), (h0, h1), (h1, S)]
                    else:
                        h = (S // 2 + 3) & ~3
                        parts = [(0, h), (h, S)]
                elif r == ROUNDS - 1 and S <= CAP:
                    if tail_mode == "narrow":
                        h = max(4, (S - 128) & ~3)
                    else:
                        h = (S // 2 + 3) & ~3
                    parts = [(0, h), (h, S)]
                elif (r == ROUNDS - 2 and S <= CAP
                      and tail_mode == "half2"):
                    h = (S // 2 + 3) & ~3
                    parts = [(0, h), (h, S)]
                else:
                    n = -(-S // CAP)
                    w = ((-(-S // n)) + 3) // 4 * 4
                    parts = [(i * w, min((i + 1) * w, S)) for i in range(n)]
                chunks.extend((r, a, b) for a, b in parts)

            # Emit every input DMA up front, in chunk order, so the SP HWDGE
            # FIFO is a pure input stream: an output DMA waiting on compute
            # can never head-of-line-block the next chunks' inputs. Tile's
            # pool slot-waits (bufs) throttle how far the inputs run ahead.
            xts = []
            for ci, (r, a, b) in enumerate(chunks):
                S = S_list[r]
                PR = P_LIST[r]
                W = b - a
                xt = xpool.tile([P, 3 * W], F32, tag="xt")
                for c in range(3):
                    # First chunk's x1 rides the ACT ring (ahead of coef) so
                    # the three planes generate in parallel across both
                    # HWDGE rings and the first compute chain starts sooner.
                    if ci == 0 and first_chunk_swdge:
                        eng = nc.gpsimd
                    elif ci == 0 and c == 1 and first_chunk_split_rings:
                        eng = nc.scalar
                    else:
                        eng = nc.sync
                    eng.dma_start(
                        out=xt[:PR, c * W : (c + 1) * W],
                        in_=xp.ap()[xbases[r] + c * PR * S : xbases[r] +
                                    (c + 1) * PR * S]
                        .rearrange("(p s) -> p s", p=PR)[:, a:b],
                    )
                xts.append(xt)
            if compute_mode == "pe":
                _pe_body(nc, tc, pool, xpool, ctile, xts, chunks, S_list,
                         obases, op, out_engine)
                _legalize_single_wait(nc)
                return nc

            for ci, (r, a, b) in enumerate(chunks):
                S = S_list[r]
                PR = P_LIST[r]
                W = b - a

                def sc(j, r=r, PR=PR):
                    return ctile[:PR, r * 16 + j : r * 16 + j + 1]

                xt = xts[ci]
                x0 = xt[:PR, 0 * W : 1 * W]
                x1 = xt[:PR, 1 * W : 2 * W]
                x2 = xt[:PR, 2 * W : 3 * W]

                un = pool.tile([P, W], F32, tag="un")
                vn = pool.tile([P, W], F32, tag="vn")
                zz = pool.tile([P, W], F32, tag="zz")
                rz = pool.tile([P, W], F32, tag="rz")
                uv = pool.tile([P, 2 * W], F32, tag="uv")

                # depth z first: it has the longest dependent chain
                # (z -> ln -> exp -> both final multiplies).
                nc.scalar.activation(out=zz[:PR], in_=x0, func=ident,
                                     scale=sc(8), bias=sc(11))
                zmid.scalar_tensor_tensor(out=zz[:PR], in0=x1, scalar=sc(9),
                                          in1=zz[:PR], op0=mult, op1=add)
                zt = nc.gpsimd if z_tail_engine == "pool" else nc.vector
                zt.scalar_tensor_tensor(out=zz[:PR], in0=x2,
                                        scalar=sc(10), in1=zz[:PR],
                                        op0=mult, op1=add)
                if recip_mode == "expln":
                    # 1/z = exp(-ln z); z in [~5, 15], ACT tables are ~2 ULP.
                    lnz = pool.tile([P, W], F32, tag="lnz")
                    nc.scalar.activation(out=lnz[:PR], in_=zz[:PR],
                                         func=mybir.ActivationFunctionType.Ln)
                    nc.scalar.activation(out=rz[:PR], in_=lnz[:PR],
                                         func=mybir.ActivationFunctionType.Exp,
                                         scale=-1.0)
                else:
                    nc.vector.reciprocal(out=rz[:PR], in_=zz[:PR])

                # u numerator: (x0*c0 + c3) + x1*c1 + x2*c2
                nc.scalar.activation(out=un[:PR], in_=x0, func=ident,
                                     scale=sc(0), bias=sc(3))
                mid.scalar_tensor_tensor(out=un[:PR], in0=x1, scalar=sc(1),
                                         in1=un[:PR], op0=mult, op1=add)
                ut = nc.gpsimd if u_tail_engine == "pool" else nc.vector
                ut.scalar_tensor_tensor(out=un[:PR], in0=x2,
                                        scalar=sc(2), in1=un[:PR],
                                        op0=mult, op1=add)
                # v numerator
                nc.scalar.activation(out=vn[:PR], in_=x0, func=ident,
                                     scale=sc(4), bias=sc(7))
                mid.scalar_tensor_tensor(out=vn[:PR], in0=x1, scalar=sc(5),
                                         in1=vn[:PR], op0=mult, op1=add)
                vt = nc.gpsimd if v_tail_engine == "pool" else nc.vector
                vt.scalar_tensor_tensor(out=vn[:PR], in0=x2,
                                        scalar=sc(6), in1=vn[:PR],
                                        op0=mult, op1=add)

                last = ci == len(chunks) - 1 and last_mul_dve
                ue = nc.gpsimd if (u_mul_engine == "pool" and not last) else nc.vector
                ve = nc.gpsimd if (v_mul_engine == "pool" and not last) else nc.vector
                ue.tensor_mul(out=uv[:PR, 0:W], in0=un[:PR], in1=rz[:PR])
                ve.tensor_mul(out=uv[:PR, W : 2 * W], in0=vn[:PR],
                              in1=rz[:PR])

                # Outputs split per plane so the u plane streams out while
                # the v multiply is still running; on out_engine's ring so
                # input DMAs never head-of-line block behind them.
                ob = obases[r]
                oview = op.ap()[ob : ob + 2 * PR * S].rearrange(
                    "(c p s) -> c p s", c=2, p=PR
                )
                oeng = getattr(nc, out_engine)
                oeng.dma_start(out=oview[0][:, a:b], in_=uv[:PR, 0:W])
                oeng.dma_start(out=oview[1][:, a:b], in_=uv[:PR, W : 2 * W])

    _legalize_single_wait(nc)
    return nc




def _pe_body(nc, tc, pool, xpool, ctile, xts, chunks, S_list, obases, op,
             out_engine):
    """PE-diagonal compute: diag(c) matmuls accumulate the three dot
    products in PSUM (per-partition scalar MAC on the otherwise idle
    TensorE); ACT does 1/z = exp(-ln z) with the z bias folded into the Ln
    op; DVE does one fused (psum + bias) * rz op per output plane."""
    mult = mybir.AluOpType.mult
    add = mybir.AluOpType.add
    # coefficient order per output: (x0, x1, x2) columns of ctile
    ucols, vcols, zcols = (0, 1, 2), (4, 5, 6), (8, 9, 10)

    with (
        tc.tile_pool(name="dpool", bufs=18) as dpool,
        tc.tile_pool(name="ppool", bufs=2, space="PSUM") as ppool,
    ):
        # per-round diagonal weight tiles, built on GPSIMD via affine_select
        diags = {}
        for r in range(ROUNDS):
            PR = P_LIST[r]
            for j in ucols + vcols + zcols:
                d = dpool.tile([P, P], F32, tag="diag")
                cb = ctile[:PR, r * 16 + j : r * 16 + j + 1].to_broadcast(
                    [PR, P]
                )
                nc.gpsimd.affine_select(
                    out=d[:PR], in_=cb, pattern=[[-1, P]],
                    channel_multiplier=1, base=0,
                    compare_op=mybir.AluOpType.is_equal, fill=0.0,
                )
                diags[(r, j)] = d

        for ci, (r, a, b) in enumerate(chunks):
            S = S_list[r]
            PR = P_LIST[r]
            W = b - a

            def sc(j, r=r, PR=PR):
                return ctile[:PR, r * 16 + j : r * 16 + j + 1]

            xt = xts[ci]
            xs = [xt[:PR, c * W : (c + 1) * W] for c in range(3)]

            pz = ppool.tile([P, W], F32, tag="pz")
            pun = ppool.tile([P, W], F32, tag="pun")
            pvn = ppool.tile([P, W], F32, tag="pvn")
            lnz = pool.tile([P, W], F32, tag="lnz")
            rz = pool.tile([P, W], F32, tag="rz")
            uv = pool.tile([P, 2 * W], F32, tag="uv")

            for k, j in enumerate(zcols):
                nc.tensor.matmul(pz[:PR], diags[(r, j)][:PR, :PR], xs[k],
                                 start=(k == 0), stop=(k == 2))
            nc.scalar.activation(out=lnz[:PR], in_=pz[:PR],
                                 func=mybir.ActivationFunctionType.Ln,
                                 bias=sc(11))
            nc.scalar.activation(out=rz[:PR], in_=lnz[:PR],
                                 func=mybir.ActivationFunctionType.Exp,
                                 scale=-1.0)
            for k, j in enumerate(ucols):
                nc.tensor.matmul(pun[:PR], diags[(r, j)][:PR, :PR], xs[k],
                                 start=(k == 0), stop=(k == 2))
            for k, j in enumerate(vcols):
                nc.tensor.matmul(pvn[:PR], diags[(r, j)][:PR, :PR], xs[k],
                                 start=(k == 0), stop=(k == 2))
            nc.vector.scalar_tensor_tensor(out=uv[:PR, 0:W], in0=pun[:PR],
                                           scalar=sc(3), in1=rz[:PR],
                                           op0=add, op1=mult)
            nc.vector.scalar_tensor_tensor(out=uv[:PR, W : 2 * W],
                                           in0=pvn[:PR], scalar=sc(7),
                                           in1=rz[:PR], op0=add, op1=mult)

            ob = obases[r]
            oview = op.ap()[ob : ob + 2 * PR * S].rearrange(
                "(c p s) -> c p s", c=2, p=PR
            )
            oeng = getattr(nc, out_engine)
            oeng.dma_start(out=oview[0][:, a:b], in_=uv[:PR, 0:W])
            oeng.dma_start(out=oview[1][:, a:b], in_=uv[:PR, W : 2 * W])

def _plan(cam_all, B):
    """Per-core camera->slot layout + global round widths."""
    npts = B // NCORES
    plans = []
    S_dev_max = np.zeros(ROUNDS, np.int64)
    bounds = np.cumsum([0] + P_LIST)
    for d in range(NCORES):
        idx = cam_all[d * npts : (d + 1) * npts]
        cnt = np.bincount(idx, minlength=NUM_CAMERAS)
        order = np.argsort(-cnt, kind="stable")  # cameras, descending count
        perm = np.argsort(idx, kind="stable")    # points grouped by camera id
        starts = np.zeros(NUM_CAMERAS, np.int64)
        starts[1:] = np.cumsum(cnt)[:-1]
        round_of = np.empty(NUM_CAMERAS, np.int64)
        part_of = np.empty(NUM_CAMERAS, np.int64)
        for r in range(ROUNDS):
            cams_r = order[bounds[r] : bounds[r + 1]]
            round_of[cams_r] = r
            part_of[cams_r] = np.arange(len(cams_r))
            S_dev_max[r] = max(S_dev_max[r], int(cnt[cams_r].max()))
        plans.append(dict(idx=idx, order=order, perm=perm, starts=starts,
                          round_of=round_of, part_of=part_of))
    S_list = [max(4, int(-(-s // 4) * 4)) for s in S_dev_max]
    return plans, S_list


def kernel(X_world, camera_indices, noisy_K, noisy_R, noisy_t,
           intrinsic_deltas, rotation_deltas, translation_deltas):
    global LAST_RESULTS

    X_world = np.asarray(X_world, dtype=np.float32)
    cam_all = np.asarray(camera_indices).astype(np.int64)
    B = X_world.shape[0]
    assert B % NCORES == 0
    npts = B // NCORES

    C = _camera_coeffs(
        np.asarray(noisy_K, np.float32), np.asarray(noisy_R, np.float32),
        np.asarray(noisy_t, np.float32), np.asarray(intrinsic_deltas, np.float32),
        np.asarray(rotation_deltas, np.float32),
        np.asarray(translation_deltas, np.float32),
    )

    plans, S_list = _plan(cam_all, B)
    global LAST_S_LIST
    LAST_S_LIST = S_list
    S_arr = np.asarray(S_list, np.int64)
    P_arr = np.asarray(P_LIST, np.int64)
    xbase = np.zeros(ROUNDS, np.int64)
    obase = np.zeros(ROUNDS, np.int64)
    for r in range(1, ROUNDS):
        xbase[r] = xbase[r - 1] + 3 * P_LIST[r - 1] * S_list[r - 1]
        obase[r] = obase[r - 1] + 2 * P_LIST[r - 1] * S_list[r - 1]
    totx = int(xbase[-1] + 3 * P_LIST[-1] * S_list[-1])
    toto = int(obase[-1] + 2 * P_LIST[-1] * S_list[-1])

    # ---- pack per-core inputs ----
    in_maps = []
    for d in range(NCORES):
        p = plans[d]
        Xd = X_world[d * npts : (d + 1) * npts]
        S_of = S_arr[p["round_of"]]
        plane_of = P_arr[p["round_of"]] * S_of
        xb0 = xbase[p["round_of"]] + p["part_of"] * S_of
        cam_sorted = p["idx"][p["perm"]]
        j_sorted = np.arange(npts, dtype=np.int64) - p["starts"][cam_sorted]
        a0 = xb0[cam_sorted] + j_sorted
        plane = plane_of[cam_sorted]
        xp = np.zeros(totx, np.float32)
        Xs = Xd[p["perm"]]
        xp[a0] = Xs[:, 0]
        xp[a0 + plane] = Xs[:, 1]
        xp[a0 + 2 * plane] = Xs[:, 2]

        coef_d = np.zeros((ROUNDS * P, 16), np.float32)
        coef_d[:, 11] = 1.0  # unused slots: z=1 (no inf/nan in pad lanes)
        slot_rows = p["round_of"] * P + p["part_of"]
        coef_d[slot_rows, :12] = C
        in_maps.append({"xp": xp, "coef": coef_d})

        p["cam_sorted"] = cam_sorted
        p["j_sorted"] = j_sorted

    # ---- build + run on the 8 NeuronCores ----
    # Retry once on transient device faults (NRT_EXEC_UNIT_UNRECOVERABLE has
    # been observed sporadically on the axon path; the identical NEFF
    # succeeds on re-execution).
    nc = _build_program(S_list, **BEST_CONFIG)
    try:
        res = run_bass_kernel_spmd(nc, in_maps, list(range(NCORES)))
    except Exception:
        res = run_bass_kernel_spmd(nc, in_maps, list(range(NCORES)))
    LAST_RESULTS = res

    # ---- unscatter back to original point order ----
    out = np.empty((B, 2), np.float32)
    for d in range(NCORES):
        p = plans[d]
        S_of = S_arr[p["round_of"]]
        ub = obase[p["round_of"]] + p["part_of"] * S_of
        ua = ub[p["cam_sorted"]] + p["j_sorted"]
        va = ua + (P_arr[p["round_of"]] * S_of)[p["cam_sorted"]]
        opd = res.results[d]["op"]
        od = out[d * npts : (d + 1) * npts]
        od[p["perm"], 0] = opd[ua]
        od[p["perm"], 1] = opd[va]
    return out

